# revision 1
# baseline (speedup 1.0000x reference)
"""JumpODE (RK4 neural-ODE + GRU + LayerNorm scan) Trainium2 Bass kernel.

Layout strategy ("F-major"): activations live as [features(partition), batch(free)]
so every matmul is lhsT=weightT tile, rhs=activation, out in PSUM, with no runtime
transposes. Batch B=512 is sharded 8 ways (64/core). The T=512 scan is sequential;
per chunk of CH steps we precompute the x-only matmuls (ODE 'base' and GRU 'gi')
as batched N=CH*64 matmuls, then run the CH sequential steps.

Host-side prep (inside kernel()): weight transposes/packing, slopes (x_t - x_{t-1})/dt,
per-step d/2,d rows, input transposition to [C,3,T,Bc] and output [H,T,Bc]->[B,T,H].
"""

import sys, os
sys.path.insert(0, "/opt/trn_rl_repo")
import numpy as np
from contextlib import ExitStack

import concourse.bass as bass
import concourse.bacc as bacc
import concourse.mybir as mybir
from concourse import tile
from concourse.bass_utils import run_bass_kernel_spmd

B, T, C, H = 512, 512, 128, 128
NCORES = 8
BC = B // NCORES          # 64 batch per core
CH = 4                    # scan steps per loop chunk
NCH = T // CH
J4 = 4 * H                # 512
FP32 = mybir.dt.float32
AF = mybir.ActivationFunctionType
ALU = mybir.AluOpType

_CACHE = {}


def _build():
    nc = bacc.Bacc("TRN2", target_bir_lowering=False, debug=False,
                   enable_asserts=False, num_devices=NCORES)
    # ---- DRAM tensors (per-core shapes) ----
    xc_d   = nc.dram_tensor("xc",   [C, 3, T, BC], FP32, kind="ExternalInput").ap()
    dr_d   = nc.dram_tensor("dr",   [T, 128],      FP32, kind="ExternalInput").ap()
    w1ut_d = nc.dram_tensor("w1ut", [128, 512],    FP32, kind="ExternalInput").ap()
    w1xt_d = nc.dram_tensor("w1xt", [128, 3, 512], FP32, kind="ExternalInput").ap()
    w2t_d  = nc.dram_tensor("w2t",  [128, 512],    FP32, kind="ExternalInput").ap()
    wih_d  = nc.dram_tensor("wih",  [128, 384],    FP32, kind="ExternalInput").ap()
    whh_d  = nc.dram_tensor("whh",  [128, 384],    FP32, kind="ExternalInput").ap()
    bias_d = nc.dram_tensor("biasr",[8, 512],      FP32, kind="ExternalInput").ap()
    gb_d   = nc.dram_tensor("gb",   [2, 256],      FP32, kind="ExternalInput").ap()
    cols_d = nc.dram_tensor("cols", [128, 4],      FP32, kind="ExternalInput").ap()
    out_d  = nc.dram_tensor("outp", [H, T, BC],    FP32, kind="ExternalOutput").ap()

    with tile.TileContext(nc) as tc, ExitStack() as ctx:
        wp   = ctx.enter_context(tc.tile_pool(name="weights", bufs=1))
        stp  = ctx.enter_context(tc.tile_pool(name="state",   bufs=1))
        xp   = ctx.enter_context(tc.tile_pool(name="xin",     bufs=2))
        prep = ctx.enter_context(tc.tile_pool(name="prec",    bufs=2))
        work = ctx.enter_context(tc.tile_pool(name="work",    bufs=3))
        outp = ctx.enter_context(tc.tile_pool(name="outst",   bufs=2))
        psA  = ctx.enter_context(tc.tile_pool(name="psA", bufs=1, space="PSUM"))  # precompute
        psU  = ctx.enter_context(tc.tile_pool(name="psU", bufs=2, space="PSUM"))  # u / gh
        psS  = ctx.enter_context(tc.tile_pool(name="psS", bufs=1, space="PSUM"))  # k / stats / bcast / dpanel

        # ---- resident weights ----
        w1ut = wp.tile([128, 512], FP32); nc.sync.dma_start(w1ut[:], w1ut_d[:])
        w1xt = wp.tile([128, 3, 512], FP32); nc.sync.dma_start(w1xt[:], w1xt_d[:])
        w2t  = wp.tile([128, 512], FP32); nc.sync.dma_start(w2t[:],  w2t_d[:])
        wih  = wp.tile([128, 384], FP32); nc.sync.dma_start(wih[:],  wih_d[:])
        whh  = wp.tile([128, 384], FP32); nc.sync.dma_start(whh[:],  whh_d[:])
        b1r  = wp.tile([1, 512], FP32);   nc.sync.dma_start(b1r[:],  bias_d[0:1, :])
        bgr  = wp.tile([1, 512], FP32);   nc.sync.dma_start(bgr[:],  bias_d[1:2, :])
        onesr= wp.tile([1, 512], FP32);   nc.sync.dma_start(onesr[:], bias_d[2:3, :])
        gbt  = wp.tile([2, 256], FP32);   nc.sync.dma_start(gbt[:],  gb_d[:])
        colt = wp.tile([128, 4], FP32);   nc.sync.dma_start(colt[:], cols_d[:])

        hP   = stp.tile([128, 64], FP32)   # current hidden state, F-major
        nc.vector.memset(hP[:], 0.0)
        brow = stp.tile([2, 128], FP32)    # bcast rhs staging: row0=[rstd|nmr], row1=[0|1]
        nc.sync.dma_start(brow[1:2, :], bias_d[4:5, 0:128])

        ones_row = onesr
        b2col    = colt[:, 0:1]      # [128,1]
        rcol     = colt[:, 1:2]      # [128,1] of 1/128
        zcol     = colt[:, 3:4]      # [128,1] zeros (activation bias)

        def layernorm(x_sl, st, which, y_out):
            """x_sl=[128,64] slice inside st=[128,128] stats tile (x at [:,0:64]).
            which: 0=ode LN, 1=gru LN. Writes normalized y to y_out [128,64]."""
            nc.vector.tensor_mul(st[:, 64:128], x_sl, x_sl)
            sps = psS.tile([1, 128], FP32, tag="stat")
            nc.tensor.matmul(sps[:, 0:128], rcol, st[:, 0:128], start=True, stop=True)
            srow = work.tile([1, 128], FP32, tag="srow")
            nc.scalar.copy(srow[:], sps[:])                       # [m | msq]
            t1 = work.tile([1, 64], FP32, tag="ln1")
            nc.vector.tensor_mul(t1[:], srow[:, 0:64], srow[:, 0:64])      # m^2
            t2 = work.tile([1, 64], FP32, tag="ln2")
            nc.vector.tensor_sub(t2[:], srow[:, 64:128], t1[:])           # var
            t3 = work.tile([1, 64], FP32, tag="ln3")
            nc.scalar.activation(t3[:], t2[:], AF.Sqrt, bias=colt[0:1, 2:3])        # sqrt(var+eps)
            t4 = work.tile([1, 64], FP32, tag="ln4")
            nc.vector.reciprocal_approx_accurate(brow[0:1, 0:64], t3[:], t4[:])  # rstd
            # nmr = -m*rstd
            nc.vector.scalar_tensor_tensor(brow[0:1, 64:128], srow[:, 0:64], -1.0,
                                           brow[0:1, 0:64], op0=ALU.mult, op1=ALU.mult)
            sb = psS.tile([128, 128], FP32, tag="bcast")
            nc.tensor.matmul(sb[:], gbt[:, which * 128:(which + 1) * 128], brow[:],
                             start=True, stop=True)               # [S | B] rank-2
            ta = work.tile([128, 64], FP32, tag="lnap")
            nc.vector.tensor_mul(ta[:], x_sl, sb[:, 0:64])
            nc.vector.tensor_add(y_out, ta[:], sb[:, 64:128])

        with tc.For_i(0, NCH) as i:
            xt = xp.tile([128, 3, CH, BC], FP32)
            nc.sync.dma_start(xt[:], xc_d[:, :, bass.ds(i * CH, CH), :])
            drt = xp.tile([1, CH, 128], FP32, tag="dr")
            nc.sync.dma_start(drt[:], dr_d[bass.ds(i * CH, CH), :])

            # ---- precompute base = W1x@xi + W1l@xl + W1s@sl + b1  (F-major [j,CH,b]) ----
            bps = psA.tile([128, 4, CH, BC], FP32, tag="pre")
            for j in range(4):
                for k in range(3):
                    nc.tensor.matmul(bps[:, j, :, :], w1xt[:, k, j * 128:(j + 1) * 128],
                                     xt[:, k, :, :], start=(k == 0), stop=False)
                nc.tensor.matmul(bps[:, j, :, :], b1r[:, j * 128:(j + 1) * 128],
                                 ones_row[:, 0:CH * BC], start=False, stop=True)
            bsb = prep.tile([128, 4, CH, BC], FP32, tag="base")
            nc.scalar.copy(bsb[:], bps[:])

            # ---- precompute gi = W_ih@xi + (b_ih+b_hh) ----
            gps = psA.tile([128, 3, CH, BC], FP32, tag="pre")
            for j in range(3):
                nc.tensor.matmul(gps[:, j, :, :], wih[:, j * 128:(j + 1) * 128],
                                 xt[:, 0, :, :], start=True, stop=False)
                nc.tensor.matmul(gps[:, j, :, :], bgr[:, j * 128:(j + 1) * 128],
                                 ones_row[:, 0:CH * BC], start=False, stop=True)
            gisb = prep.tile([128, 3, CH, BC], FP32, tag="gi")
            nc.scalar.copy(gisb[:], gps[:])

            stg = outp.tile([128, CH, BC], FP32)

            for s in range(CH):
                # D panels: [d/2 | d] broadcast down partitions
                dps = psS.tile([128, 128], FP32, tag="dpan")
                nc.tensor.matmul(dps[:], ones_row[:, 0:128], drt[:, s, :],
                                 start=True, stop=True)
                dsb = work.tile([128, 128], FP32, tag="dsb")
                nc.scalar.copy(dsb[:], dps[:])

                hs = hP
                tmps = []
                for stage in range(4):
                    ups = psU.tile([128, 4, BC], FP32, tag="u")
                    for j in range(4):
                        nc.tensor.matmul(ups[:, j, :], w1ut[:, j * 128:(j + 1) * 128],
                                         hs[:], start=True, stop=True)
                    tu = work.tile([128, 4, BC], FP32, tag="tu")
                    nc.vector.tensor_add(tu[:], ups[:], bsb[:, :, s, :])
                    av = work.tile([128, 4, BC], FP32, tag="act")
                    nc.scalar.activation(av[:], tu[:], AF.Silu, bias=zcol)
                    kps = psS.tile([128, 64], FP32, tag="k")
                    for j in range(4):
                        nc.tensor.matmul(kps[:], w2t[:, j * 128:(j + 1) * 128],
                                         av[:, j, :], start=(j == 0), stop=(j == 3))
                    # tmp = (k + b2) * c_panel ; c = d/2,d/2,d,d/2
                    pan = dsb[:, 64:128] if stage == 2 else dsb[:, 0:64]
                    tmp = work.tile([128, 64], FP32, tag=f"tmp{stage}")
                    nc.vector.scalar_tensor_tensor(tmp[:], kps[:], b2col, pan,
                                                   op0=ALU.add, op1=ALU.mult)
                    tmps.append(tmp)
                    if stage < 3:
                        hn = work.tile([128, 64], FP32, tag=f"hs{stage}")
                        nc.vector.tensor_add(hn[:], hP[:], tmp[:])
                        hs = hn

                # h_pre = hP + (tmp1 + 2*tmp2 + tmp3 + tmp4)/3   (into stats tile)
                st1 = work.tile([128, 128], FP32, tag="st1")
                acc = work.tile([128, 64], FP32, tag="acc")
                nc.vector.scalar_tensor_tensor(acc[:], tmps[1], 2.0, tmps[0],
                                               op0=ALU.mult, op1=ALU.add)
                nc.vector.tensor_add(acc[:], acc[:], tmps[2])
                nc.vector.tensor_add(acc[:], acc[:], tmps[3])
                nc.vector.scalar_tensor_tensor(st1[:, 0:64], acc[:], 1.0 / 3.0, hP[:],
                                               op0=ALU.mult, op1=ALU.add)
                hA = work.tile([128, 64], FP32, tag="hA")
                layernorm(st1[:, 0:64], st1, 0, hA[:])

                # ---- GRU ----
                ghp = psU.tile([128, 3, 64], FP32, tag="u")
                for j in range(3):
                    nc.tensor.matmul(ghp[:, j, :], whh[:, j * 128:(j + 1) * 128],
                                     hA[:], start=True, stop=True)
                grz = work.tile([128, 128], FP32, tag="grz")
                nc.vector.tensor_add(grz[:], ghp[:, 0:2, :], gisb[:, 0:2, s, :])
                rz = work.tile([128, 128], FP32, tag="rz")
                nc.scalar.activation(rz[:], grz[:], AF.Sigmoid, bias=zcol)
                rhn = work.tile([128, 64], FP32, tag="rhn")
                nc.vector.tensor_mul(rhn[:], rz[:, 0:64], ghp[:, 2, :])
                nin = work.tile([128, 64], FP32, tag="nin")
                nc.vector.tensor_add(nin[:], rhn[:], gisb[:, 2, s, :])
                ntl = work.tile([128, 64], FP32, tag="ntl")
                nc.scalar.activation(ntl[:], nin[:], AF.Tanh, bias=zcol)
                # h2pre = n + z*(hA - n)
                st2 = work.tile([128, 128], FP32, tag="st2")
                td = work.tile([128, 64], FP32, tag="td")
                nc.vector.tensor_sub(td[:], hA[:], ntl[:])
                tz = work.tile([128, 64], FP32, tag="tz")
                nc.vector.tensor_mul(tz[:], rz[:, 64:128], td[:])
                nc.vector.tensor_add(st2[:, 0:64], ntl[:], tz[:])
                layernorm(st2[:, 0:64], st2, 1, hP[:])
                nc.gpsimd.tensor_scalar_add(stg[:, s, :], hP[:], 0.0)

            nc.sync.dma_start(out_d[:, bass.ds(i * CH, CH), :], stg[:])

    nc.compile()
    return nc


def _host_prep(x, ts, W1, b1, W2, b2, W_ih, W_hh, b_ih, b_hh,
               g_ode, beta_ode, g_gru, beta_gru):
    f32 = np.float32
    x = np.asarray(x, f32); ts = np.asarray(ts, f32)
    W1 = np.asarray(W1, f32)
    # global (core-independent) tensors
    w1ut = np.ascontiguousarray(W1[:, 0:128].T)                       # [128,512]
    w1xt = np.ascontiguousarray(
        np.stack([W1[:, 128 * (k + 1):128 * (k + 2)].T for k in range(3)], axis=1))  # [128,3,512]
    W2 = np.asarray(W2, f32)
    w2t = np.ascontiguousarray(
        np.stack([W2.T[j * 128:(j + 1) * 128] for j in range(4)], axis=1)
    ).reshape(128, 512)                                               # [k, j*128+m]
    wih = np.ascontiguousarray(np.asarray(W_ih, f32).T)               # [128,384]
    whh = np.ascontiguousarray(np.asarray(W_hh, f32).T)               # [128,384]
    biasr = np.zeros((8, 512), f32)
    biasr[0, :] = np.asarray(b1, f32)
    biasr[1, :384] = np.asarray(b_ih, f32) + np.asarray(b_hh, f32)
    biasr[2, :] = 1.0
    biasr[4, 64:128] = 1.0
    gb = np.zeros((2, 256), f32)
    gb[0, 0:128] = np.asarray(g_ode, f32);  gb[1, 0:128] = np.asarray(beta_ode, f32)
    gb[0, 128:256] = np.asarray(g_gru, f32); gb[1, 128:256] = np.asarray(beta_gru, f32)
    cols = np.zeros((128, 4), f32)
    cols[:, 0] = np.asarray(b2, f32)
    cols[:, 1] = 1.0 / 128.0
    cols[:, 2] = 1e-5

    # per-step dt and slopes (t=0: d=0 -> RK4 no-op; sl=0)
    dt = np.zeros((B, T), f32)
    dt[:, 1:] = ts[:, 1:] - ts[:, :-1]
    xl = np.concatenate([x[:, :1], x[:, :-1]], axis=1)                # [B,T,C]
    sl = np.zeros_like(x)
    sl[:, 1:] = (x[:, 1:] - xl[:, 1:]) / dt[:, 1:, None]

    xcat = np.stack([x, xl, sl], axis=1)                              # [B,3,T,C]
    per_core = []
    for c in range(NCORES):
        bs = slice(c * BC, (c + 1) * BC)
        xc = np.ascontiguousarray(xcat[bs].transpose(3, 1, 2, 0))     # [C,3,T,Bc]
        d = dt[bs]                                                    # [Bc,T]
        dr = np.zeros((T, 128), f32)
        dr[:, 0:64] = 0.5 * d.T
        dr[:, 64:128] = d.T
        per_core.append({
            "xc": xc, "dr": dr, "w1ut": w1ut, "w1xt": w1xt, "w2t": w2t,
            "wih": wih, "whh": whh, "biasr": biasr, "gb": gb, "cols": cols,
        })
    return per_core


def kernel(**inputs):
    if "nc" not in _CACHE:
        _CACHE["nc"] = _build()
    nc = _CACHE["nc"]
    in_maps = _host_prep(**inputs)
    res = run_bass_kernel_spmd(nc, in_maps, list(range(NCORES)))
    outs = []
    for c in range(NCORES):
        o = res.results[c]["outp"]                                    # [H,T,Bc]
        outs.append(np.ascontiguousarray(o.transpose(2, 1, 0)))       # [Bc,T,H]
    return np.concatenate(outs, axis=0).astype(np.float32)            # [B,T,H]

